# revision 2
# baseline (speedup 1.0000x reference)
"""GraphConvolution kernel for Trainium2 (8 NeuronCores, SPMD).

out = segment_sum(edge_w * (x @ W)[edge_src], edge_dst) + b

Strategy (graph/data parallel, dst-sharded):
  - Each core owns a contiguous shard of 12500 destination nodes, split
    into 98 dst blocks of 128. W commutes with segment_sum, so each core
    gathers raw x rows, accumulates pre[d, :] = sum_e w_e * x[src_e, :]
    per dst block via selection-matrix matmuls in PSUM (f32), then
    applies W per block and adds the bias.
  - The gather uses dma_gather (GPSIMD extended instruction, int16
    indices). Indices are int16, so x is addressed through 4 quartile
    tables of 25000 rows. Edges are bucketed per (dst block, quartile)
    cell; every cell is padded to a fixed chunk capacity so all 8 cores
    run one identical SPMD instruction stream. Pad slots gather row 0 of
    the quartile with weight 0.
  - The kernel is Q7-descriptor-rate bound (~8-9 ns/slot); all other
    engines (PE matmuls, DVE selection-matrix builds, ACT copies, DMA
    stores) hide underneath the gather.
"""

import numpy as np

import concourse.bass as bass
import concourse.bacc as bacc
import concourse.mybir as mybir
import concourse.tile as tile
from concourse.bass_utils import run_bass_kernel_spmd
from concourse.masks import make_identity

N_NODES = 100000
D_IN = 256
D_OUT = 128
N_CORES = 8
SHARD = N_NODES // N_CORES          # 12500 dst rows per core
P = 128
NBLK = (SHARD + P - 1) // P         # 98 dst blocks per core
OUT_ROWS = NBLK * P                 # 12544 padded output rows per core
NQ = 4
QROWS = (N_NODES + NQ - 1) // NQ    # 25000 rows per quartile table

last_exec_time_ns = None
_program_cache = {}


def _build_program(cq):
    """One SPMD program: NBLK dst blocks x (NQ x cq) chunks of 128 edges."""
    f32 = mybir.dt.float32
    i16 = mybir.dt.int16
    C = NQ * cq                      # chunks per dst block
    call_idx = cq * P                # indices per dma_gather call
    ncalls = NBLK * NQ
    icols = call_idx // 16           # int16 idx columns per call

    nc = bacc.Bacc("TRN2", target_bir_lowering=False, debug=False,
                   num_devices=N_CORES)
    x_tbl = nc.dram_tensor("x_tbl", [N_NODES, D_IN], f32,
                           kind="ExternalInput").ap()
    wmat = nc.dram_tensor("wmat", [D_IN, D_OUT], f32, kind="ExternalInput").ap()
    bbc = nc.dram_tensor("bbc", [P, D_OUT], f32, kind="ExternalInput").ap()
    idx = nc.dram_tensor("idx", [P, ncalls * icols], i16,
                         kind="ExternalInput").ap()
    mdst = nc.dram_tensor("mdst", [P, NBLK * C], f32, kind="ExternalInput").ap()
    mw = nc.dram_tensor("mw", [P, NBLK * C], f32, kind="ExternalInput").ap()
    out = nc.dram_tensor("out", [OUT_ROWS, D_OUT], f32,
                         kind="ExternalOutput").ap()

    with tile.TileContext(nc) as tc:
        with (
            tc.tile_pool(name="const", bufs=1) as constp,
            tc.tile_pool(name="meta", bufs=1) as metap,
            tc.tile_pool(name="g", bufs=3) as gp,
            tc.tile_pool(name="m", bufs=8) as mp,
            tc.tile_pool(name="pre", bufs=2, space="PSUM") as prep,
            tc.tile_pool(name="tp", bufs=2, space="PSUM") as tpp,
            tc.tile_pool(name="po", bufs=2, space="PSUM") as pop,
            tc.tile_pool(name="sb", bufs=3) as sbp,
            tc.tile_pool(name="st", bufs=4) as stp,
            tc.tile_pool(name="ob", bufs=3) as obp,
        ):
            w0 = constp.tile([P, D_OUT], f32, tag="w0")
            w1 = constp.tile([P, D_OUT], f32, tag="w1")
            nc.sync.dma_start(out=w0[:], in_=wmat[0:P, :])
            nc.sync.dma_start(out=w1[:], in_=wmat[P:2 * P, :])
            bb = constp.tile([P, D_OUT], f32, tag="bb")
            nc.sync.dma_start(out=bb[:], in_=bbc[:])
            iota_i = constp.tile([P, P], mybir.dt.int32, tag="ioi")
            nc.gpsimd.iota(iota_i[:], pattern=[[1, P]], base=0,
                           channel_multiplier=0)
            iota_f = constp.tile([P, P], f32, tag="iof")
            nc.vector.tensor_copy(iota_f[:], iota_i[:])
            ident = constp.tile([P, P], f32, tag="id")
            make_identity(nc, ident[:])

            idx_t = metap.tile([P, ncalls * icols], i16, tag="idx")
            mdst_t = metap.tile([P, NBLK * C], f32, tag="mdst")
            mw_t = metap.tile([P, NBLK * C], f32, tag="mw")
            nc.sync.dma_start(out=idx_t[:], in_=idx[:])
            nc.sync.dma_start(out=mdst_t[:], in_=mdst[:])
            nc.sync.dma_start(out=mw_t[:], in_=mw[:])

            for b in range(NBLK):
                gt = gp.tile([P, C * D_IN], f32, tag="g")
                for q in range(NQ):
                    call = b * NQ + q
                    nc.gpsimd.dma_gather(
                        out_ap=gt[:, q * cq * D_IN:(q + 1) * cq * D_IN]
                        .rearrange("p (c d) -> p c d", d=D_IN),
                        in_ap=x_tbl[q * QROWS:(q + 1) * QROWS, :],
                        idxs_ap=idx_t[:, call * icols:(call + 1) * icols],
                        num_idxs=call_idx, num_idxs_reg=call_idx,
                        elem_size=D_IN, single_packet=False,
                    )
                pre = prep.tile([P, D_IN], f32, tag="pre")
                for c in range(C):
                    col = b * C + c
                    mt = mp.tile([P, P], f32, tag="m")
                    nc.vector.tensor_scalar(
                        out=mt[:], in0=iota_f[:],
                        scalar1=mdst_t[:, col:col + 1],
                        scalar2=mw_t[:, col:col + 1],
                        op0=mybir.AluOpType.is_equal,
                        op1=mybir.AluOpType.mult,
                    )
                    nc.tensor.matmul(
                        out=pre[:], lhsT=mt[:],
                        rhs=gt[:, c * D_IN:(c + 1) * D_IN],
                        start=(c == 0), stop=(c == C - 1),
                    )
                sb_pre = sbp.tile([P, D_IN], f32, tag="sb")
                nc.scalar.copy(sb_pre[:], pre[:])
                po = pop.tile([P, D_OUT], f32, tag="po")
                for h in range(2):
                    pt = tpp.tile([P, P], f32, tag="pt")
                    nc.tensor.transpose(pt[:], sb_pre[:, h * P:(h + 1) * P],
                                        ident[:])
                    st = stp.tile([P, P], f32, tag="st")
                    nc.scalar.copy(st[:], pt[:])
                    nc.tensor.matmul(out=po[:], lhsT=st[:],
                                     rhs=(w0[:] if h == 0 else w1[:]),
                                     start=(h == 0), stop=(h == 1))
                ob = obp.tile([P, D_OUT], f32, tag="ob")
                nc.vector.tensor_add(ob[:], po[:], bb[:])
                nc.sync.dma_start(out=out[b * P:(b + 1) * P, :], in_=ob[:])

    nc.compile()
    return nc


def _prep_inputs(x, edge_src, edge_dst, edge_w, W, b):
    """Bucket edges per (core, dst block, src quartile) cell, pad each cell
    to a uniform chunk capacity cq, and build the device-side idx/meta
    arrays. Pads gather row 0 of their quartile with weight 0."""
    edge_src = np.asarray(edge_src, np.int64)
    edge_dst = np.asarray(edge_dst, np.int64)
    edge_w = np.asarray(edge_w, np.float32)

    core = edge_dst // SHARD
    loc = edge_dst - core * SHARD
    blk = loc >> 7
    dst_local = (loc & 127).astype(np.float32)
    q = edge_src // QROWS
    src_local = (edge_src - q * QROWS).astype(np.int16)

    cell = ((core * NBLK + blk) * NQ + q)
    ncell = N_CORES * NBLK * NQ
    order = np.argsort(cell, kind="stable")
    cell_s = cell[order]
    counts = np.bincount(cell_s, minlength=ncell)
    cq = int(max(1, (counts.max() + P - 1) // P))
    cap = cq * P

    starts = np.zeros(ncell, np.int64)
    starts[1:] = np.cumsum(counts)[:-1]
    rank = np.arange(len(order)) - starts[cell_s]

    C = NQ * cq
    # int16 gather indices, per call (b, q), wrapped [16, cap/16] layout
    idx_all = np.zeros((N_CORES, NBLK * NQ, cap), np.int16)
    core_s = cell_s // (NBLK * NQ)
    rem = cell_s - core_s * (NBLK * NQ)          # b * NQ + q
    idx_all[core_s, rem, rank] = src_local[order]
    icols = cap // 16
    # position j -> [j % 16, j // 16], replicated over 8 partition groups
    idx_wrapped = idx_all.reshape(N_CORES, NBLK * NQ, icols, 16)
    idx_wrapped = idx_wrapped.transpose(0, 3, 1, 2).reshape(N_CORES, 16, -1)
    idx_wrapped = np.tile(idx_wrapped, (1, 8, 1))

    # per-slot metadata: column = b * C + q * cq + rank // 128, partition =
    # rank % 128
    mdst_all = np.zeros((N_CORES, P, NBLK * C), np.float32)
    mw_all = np.zeros((N_CORES, P, NBLK * C), np.float32)
    b_s = rem // NQ
    q_s = rem - b_s * NQ
    colpos = b_s * C + q_s * cq + (rank >> 7)
    part = rank & 127
    mdst_all[core_s, part, colpos] = dst_local[order]
    mw_all[core_s, part, colpos] = edge_w[order]

    bbc = np.broadcast_to(np.asarray(b, np.float32), (P, D_OUT)).copy()
    wmat = np.ascontiguousarray(np.asarray(W, np.float32))
    x_tbl = np.ascontiguousarray(np.asarray(x, np.float32))

    in_maps = []
    for m in range(N_CORES):
        in_maps.append({
            "x_tbl": x_tbl,
            "wmat": wmat,
            "bbc": bbc,
            "idx": np.ascontiguousarray(idx_wrapped[m]),
            "mdst": mdst_all[m],
            "mw": mw_all[m],
        })
    return in_maps, cq


def kernel(x, edge_src, edge_dst, edge_w, W, b):
    global last_exec_time_ns
    in_maps, cq = _prep_inputs(x, edge_src, edge_dst, edge_w, W, b)
    if cq not in _program_cache:
        _program_cache[cq] = _build_program(cq)
    nc = _program_cache[cq]
    res = run_bass_kernel_spmd(nc, in_maps, list(range(N_CORES)))
    last_exec_time_ns = res.exec_time_ns
    shards = [res.results[m]["out"][:SHARD] for m in range(N_CORES)]
    return np.concatenate(shards, axis=0).astype(np.float32)
